# revision 9
# baseline (speedup 1.0000x reference)
"""3-layer GCN + global mean pool + linear head on 8 Trainium2 NeuronCores.

Strategy (dst-sharded message passing):
  - GCN normalization factorizes: norm_e = dinv[src]*dinv[dst], so each conv
    layer is  h' = relu( dinv * ((Adj+I) @ (dinv * h)) @ W + b ).  Only pure
    row gather + segment-sum on device; diagonal scalings are per-node ops.
  - Nodes (and their in-edges) are sharded across the 8 cores by contiguous
    dst ranges.  Edges are grouped by (dst 128-block, source window) and
    padded to a uniform number of 128-edge chunks (padding uses dst_rel=-1,
    whose one-hot column is zero; padding fetch rows cycle the cell's own
    real sources to avoid single-row HBM hotspots).  Self-loops are NOT in
    the edge stream: the identity term is added per dst block with one
    transpose-by-identity matmul (this also shrinks the max-over-cores chunk
    padding from ~32% to ~19%).
  - Per layer: each core's h~ slice (dinv*h, bf16, features padded to 128 so
    rows are 256B - the dma_gather minimum) is AllGathered into a DRAM
    table; each core gathers its edges' source rows with dma_gather over 4
    SWDGE queues in 1024-token calls, each call landing in its own rotating
    SBUF tile (per-call tiles let the queues run unserialized).  Segment-sum
    runs on the TensorEngine: per 128-edge chunk, aggT[64f,128d] +=
    msgs[128e,0:64].T @ M[128e,128d], with M built by one broadcast is_equal
    against an iota tile.  The layer weight applies after aggregation, then
    dinv/bias/relu as per-partition DVE ops.
  - Layer-1 input h~ = dinv*x is precomputed on host (pure preprocessing).
    Mean-pool uses the same one-hot matmul against graph ids fused into the
    layer-3 epilogue, partials are AllReduced, head matmul on every core.

Host-side work is sharding-style preprocessing only: edge sort/group/pad,
degree bincount (dinv), graph-size bincount, layout permutation.
"""
import numpy as np
import ml_dtypes

P = 128
NCORES = 8
NWIN = 4          # int16 source windows
TMAX = 1024       # max tokens per dma_gather call (SWDGE ring limit)
SGBLK = 10        # dst blocks per super-group (gather batching granularity)
NQ = 4            # SWDGE queues
NT = 32           # per-call msgs tile rotation depth

# Full-size problem dims (nn_GCN_13881334300836)
N_FULL, E_FULL, D_FULL, C_FULL, G_FULL = 100_000, 1_250_000, 64, 10, 128


# --------------------------------------------------------------------------
# Host preprocessing
# --------------------------------------------------------------------------

def preprocess(x, edge_index, batch, n_cores=NCORES, sort_cells=True):
    """Shard nodes/edges; build window-grouped, chunk-padded gather indices.

    Permuted global row for node n (core, local=n-core*npc, b=local//P,
    p=local%P):  row = (core*P + p)*nblk + b.  Each core's h~ slice is then
    one contiguous SBUF->DRAM DMA and AllGather concatenation.  Window w
    covers rows [w*wrows, (w+1)*wrows), wrows = n_cores*P*nblk/NWIN.
    """
    N, H = x.shape
    assert N % n_cores == 0
    npc = N // n_cores
    nblk = -(-npc // P)
    nblk = -(-nblk // NWIN) * NWIN      # pad to a multiple of NWIN quarters
    npad = nblk * P
    QB = nblk // NWIN                   # blocks per quarter
    wrows = n_cores * P * QB            # rows per window (= quarter)
    assert wrows <= 32768

    ei = edge_index.astype(np.int64)
    src_all = ei[0]
    dst_all = ei[1]

    # in-degree including the self-loop each node gets
    deg = (np.bincount(dst_all, minlength=N) + 1).astype(np.float32)
    dinv = (1.0 / np.sqrt(deg)).astype(np.float32)

    # quarter-interleaved permuted row: window w holds the w-th quarter of
    # EVERY core's slice, so the per-layer table can be AllGathered in 4
    # quarter chunks, each ready as soon as that quarter's blocks are done
    core_of = src_all // npc
    local = src_all - core_of * npc
    b_of = local // P
    q_of = b_of // QB
    src_perm = ((q_of * n_cores + core_of) * P + local % P) * QB + b_of % QB
    win_all = src_perm // wrows

    # per-core (block, window) edge lists
    edges = []   # [core][b][w] -> (idx int16 array, drel array)
    K = np.zeros((n_cores, nblk, NWIN), np.int64)
    for c in range(n_cores):
        lo = c * npc
        m = (dst_all >= lo) & (dst_all < lo + npc)
        s = src_perm[m]
        w = win_all[m]
        d = dst_all[m] - lo
        key = (d // P) * NWIN + w
        order = np.argsort(key, kind="stable")
        s, w, d, key = s[order], w[order], d[order], key[order]
        cnt = np.bincount(key, minlength=nblk * NWIN)
        off = np.zeros(nblk * NWIN + 1, np.int64)
        np.cumsum(cnt, out=off[1:])
        percore = []
        for b in range(nblk):
            row = []
            for ww in range(NWIN):
                k = b * NWIN + ww
                sl = slice(off[k], off[k + 1])
                s_c, d_c = s[sl], d[sl]
                if sort_cells and len(s_c):
                    o_c = np.argsort(s_c, kind="stable")
                    s_c, d_c = s_c[o_c], d_c[o_c]
                row.append(((s_c - ww * wrows).astype(np.int16),
                            (d_c % P).astype(np.float32)))
                K[c, b, ww] = (cnt[k] + P - 1) // P
            percore.append(row)
        edges.append(percore)

    Kmax = K.max(axis=0)          # [nblk, NWIN] uniform chunk counts
    sgblk = SGBLK
    if nblk % sgblk != 0:
        sgblk = next((g for g in (7, 8, 6, 5, 4, 9, 10, 3, 2) if nblk % g == 0),
                     nblk)
    nsg = nblk // sgblk

    # token stream: for sg: for w: for b in sg: Kmax[b,w] chunks of 128
    # chunk positions (global column index) and per-(sg,w) call splits
    chunk_pos = np.zeros((nblk, NWIN), np.int64)   # starting chunk column
    sg_tok0 = []                                   # sg -> token start
    sg_w_ranges = []                               # sg -> [(w, tok0, tok1)]
    pos = 0
    for sg in range(nsg):
        sg_tok0.append(pos * P)
        rngs = []
        for ww in range(NWIN):
            t0 = pos * P
            for b in range(sg * sgblk, (sg + 1) * sgblk):
                chunk_pos[b, ww] = pos
                pos += Kmax[b, ww]
            rngs.append((ww, t0, pos * P))
        sg_w_ranges.append(rngs)
    nchunk = pos
    ntok = nchunk * P

    rng = np.random.default_rng(1234)
    eidx16 = np.zeros((n_cores, 16, ntok // 16), np.int16)
    edst = np.full((n_cores, P, nchunk), -1.0, np.float32)
    for c in range(n_cores):
        stream = np.zeros(ntok, np.int16)
        for b in range(nblk):
            for ww in range(NWIN):
                s16, dr = edges[c][b][ww]
                t0 = chunk_pos[b, ww] * P
                npadtok = Kmax[b, ww] * P - len(s16)
                if len(s16):
                    # padding fetches cycle the cell's own sources (spread
                    # across banks, page-local); dst_rel=-1 zeroes them out
                    fill = (np.resize(s16, npadtok) if npadtok
                            else np.empty(0, np.int16))
                else:
                    fill = rng.integers(0, wrows,
                                        size=npadtok).astype(np.int16)
                cell = np.concatenate([s16, fill])
                stream[t0:t0 + len(cell)] = cell
                # dst_rel per slot: token t -> (p=t%128, chunk=t//128)
                col = chunk_pos[b, ww]
                nchunks_b = Kmax[b, ww]
                dcols = np.full((nchunks_b * P,), -1.0, np.float32)
                dcols[:len(dr)] = dr
                edst[c][:, col:col + nchunks_b] = dcols.reshape(nchunks_b,
                                                                P).T
        eidx16[c] = stream.reshape(ntok // 16, 16).T

    dinv_pc = np.zeros((n_cores, P, nblk), np.float32)
    bat_pc = np.full((n_cores, P, nblk), -1.0, np.float32)
    ht0_pc = np.zeros((n_cores, P, nblk * P), np.float32)
    xf = np.asarray(x, np.float32)
    for c in range(n_cores):
        dv = np.zeros(npad, np.float32)
        dv[:npc] = dinv[c * npc:(c + 1) * npc]
        dinv_pc[c] = dv.reshape(nblk, P).T
        bt = np.full(npad, -1.0, np.float32)
        bt[:npc] = batch[c * npc:(c + 1) * npc].astype(np.float32)
        bat_pc[c] = bt.reshape(nblk, P).T
        ht = np.zeros((npad, P), np.float32)
        ht[:npc, :H] = xf[c * npc:(c + 1) * npc] * dv[:npc, None]
        ht0_pc[c] = ht.reshape(nblk, P, P).transpose(1, 0, 2).reshape(P, -1)

    return dict(eidx16=eidx16, edst=edst, dinv=dinv_pc, batg=bat_pc,
                ht0=ht0_pc, npc=npc, nblk=nblk, nsg=nsg, sgblk=sgblk,
                ntok=ntok, nchunk=nchunk, Kmax=Kmax, chunk_pos=chunk_pos,
                sg_tok0=sg_tok0, sg_w_ranges=sg_w_ranges, wrows=wrows, H=H)


# --------------------------------------------------------------------------
# Device kernel builder
# --------------------------------------------------------------------------

def build_nc(pp, G, C, n_cores=NCORES, reps=1, skip=frozenset(),
             shared_table=True, percall_msgs=True):
    """Build the Bass program (shared SPMD across n_cores).

    reps: repeat the whole compute body (benchmarking only; output unchanged).
    skip: ablation set — any of {"gather", "collective", "matmul"}.
    """
    import concourse.bacc as bacc
    import concourse.mybir as mybir
    import concourse.tile as tile
    from contextlib import ExitStack

    H = pp["H"]
    nblk, nsg, ntok, nchunk = pp["nblk"], pp["nsg"], pp["ntok"], pp["nchunk"]
    Kmax, chunk_pos = pp["Kmax"], pp["chunk_pos"]
    sg_tok0, sg_w_ranges, wrows = pp["sg_tok0"], pp["sg_w_ranges"], pp["wrows"]
    RG = [list(range(n_cores))]
    EL = P  # padded feature width (256B rows)

    # static call table: call k covers tokens [cs[k], cs[k+1]) of the stream
    call_bounds = []
    for sg in range(nsg):
        for (ww, t0, t1) in sg_w_ranges[sg]:
            t = t0
            while t < t1:
                tc_ = min(TMAX, t1 - t)
                call_bounds.append((t, t + tc_))
                t += tc_
    call_start_toks = np.array([b[0] for b in call_bounds])

    def chunk_to_call(g):
        """global chunk index -> (call index, slot within call tile)"""
        tok = g * P
        k = int(np.searchsorted(call_start_toks, tok, side="right")) - 1
        return k, (tok - call_bounds[k][0]) // P

    f32, bf16 = mybir.dt.float32, mybir.dt.bfloat16
    i16 = mybir.dt.int16
    AL = mybir.AluOpType

    nc = bacc.Bacc("TRN2", target_bir_lowering=False, debug=False,
                   enable_asserts=False, num_devices=n_cores,
                   num_swdge_queues=NQ)

    eidx_d = nc.dram_tensor("eidx", [16, ntok // 16], i16, kind="ExternalInput")
    edst_d = nc.dram_tensor("edst", [P, nchunk], bf16, kind="ExternalInput")
    ht0_d = nc.dram_tensor("ht0", [P, nblk * P], bf16, kind="ExternalInput")
    dinv_d = nc.dram_tensor("dinv", [P, nblk], f32, kind="ExternalInput")
    batg_d = nc.dram_tensor("batg", [P, nblk], f32, kind="ExternalInput")
    iota_bf_d = nc.dram_tensor("iota_bf", [P, P], bf16, kind="ExternalInput")
    ident_bf_d = nc.dram_tensor("ident_bf", [P, P], bf16, kind="ExternalInput")
    iota_f_d = nc.dram_tensor("iota_f", [P, P], f32, kind="ExternalInput")
    w_d = [nc.dram_tensor(f"w{l}", [H, H], f32, kind="ExternalInput")
           for l in range(3)]
    bias_d = [nc.dram_tensor(f"bias{l}", [P, H], f32, kind="ExternalInput")
              for l in range(3)]
    wl_d = nc.dram_tensor("wl", [H, C], f32, kind="ExternalInput")
    biasl_d = nc.dram_tensor("biasl", [P, C], f32, kind="ExternalInput")
    cinv_d = nc.dram_tensor("cinv", [P, 1], f32, kind="ExternalInput")
    out_d = nc.dram_tensor("out", [G, C], f32, kind="ExternalOutput")

    with tile.TileContext(nc) as tc:
        with ExitStack() as ctx:
            const = ctx.enter_context(tc.tile_pool(name="const", bufs=1))
            msgs_tp = ctx.enter_context(tc.tile_pool(name="msgs", bufs=NT))
            m_tp = ctx.enter_context(tc.tile_pool(name="mb", bufs=3))
            s_tp = ctx.enter_context(tc.tile_pool(name="st", bufs=3))
            e_tp = ctx.enter_context(tc.tile_pool(name="ep", bufs=4))
            agg_ps = ctx.enter_context(tc.tile_pool(name="aggp", bufs=4,
                                                    space="PSUM"))
            out_ps = ctx.enter_context(tc.tile_pool(name="outp", bufs=2,
                                                    space="PSUM"))
            fin_ps = ctx.enter_context(tc.tile_pool(name="finp", bufs=1,
                                                    space="PSUM"))
            dram = ctx.enter_context(tc.tile_pool(name="dram", bufs=1,
                                                  space="DRAM"))

            eidx_sb = const.tile([128, ntok // 16], i16)
            edst_sb = const.tile([P, nchunk], bf16)
            iota_bf = const.tile([P, P], bf16)
            ident_bf = const.tile([P, P], bf16)
            iota_f = const.tile([P, P], f32)
            dinv_sb = const.tile([P, nblk], f32)
            batg_sb = const.tile([P, nblk], f32)
            w_sb = [const.tile([H, H], f32, tag=f"w{l}", name=f"w{l}_sb")
                    for l in range(3)]
            bias_sb = [const.tile([P, H], f32, tag=f"b{l}", name=f"b{l}_sb")
                       for l in range(3)]
            wl_sb = const.tile([H, C], f32)
            biasl_sb = const.tile([P, C], f32)
            cinv_sb = const.tile([P, 1], f32)
            ht_sb = const.tile([P, nblk, EL], bf16)   # h~ slice, 256B rows

            # idx tile: replicate the [16, S] wrap to all 8 partition groups
            for g8 in range(8):
                nc.sync.dma_start(eidx_sb[:][g8 * 16:(g8 + 1) * 16, :],
                                  eidx_d.ap())
            nc.sync.dma_start(edst_sb[:], edst_d.ap())
            nc.sync.dma_start(iota_bf[:], iota_bf_d.ap())
            nc.sync.dma_start(ident_bf[:], ident_bf_d.ap())
            nc.sync.dma_start(iota_f[:], iota_f_d.ap())
            nc.sync.dma_start(dinv_sb[:], dinv_d.ap())
            nc.sync.dma_start(batg_sb[:], batg_d.ap())
            for l in range(3):
                nc.sync.dma_start(w_sb[l][:], w_d[l].ap())
                nc.sync.dma_start(bias_sb[l][:], bias_d[l].ap())
            nc.sync.dma_start(wl_sb[:], wl_d.ap())
            nc.sync.dma_start(biasl_sb[:], biasl_d.ap())
            nc.sync.dma_start(cinv_sb[:], cinv_d.ap())

            QB = nblk // NWIN
            in_ccq = [dram.tile([P, QB * EL], bf16, tag=f"incc{q}",
                                name=f"incc{q}") for q in range(NWIN)]
            hfq_r = [[[dram.tile([n_cores * P, QB * EL], bf16,
                                 addr_space=("Shared" if shared_table
                                             else "Local"),
                                 tag=f"hf{l}_{q}_{r}",
                                 name=f"hf{l}_{q}_{r}")
                       for q in range(NWIN)] for l in range(3)]
                      for r in range(reps)]
            prd_in = dram.tile([H, P], f32)
            prd_out_r = [dram.tile([H, P], f32, addr_space="Shared",
                                   tag=f"prd_out_{r}", name=f"prd_out_{r}")
                         for r in range(reps)]

            def emit_cc(q, hf):
                # ship quarter q of next-layer h~ while other blocks compute
                nc.sync.dma_start(
                    in_ccq[q][:],
                    ht_sb[:][:, q * QB:(q + 1) * QB, :]
                        .rearrange("p b e -> p (b e)"))
                if "collective" not in skip:
                    nc.gpsimd.collective_compute(
                        "AllGather", AL.bypass, replica_groups=RG,
                        ins=[in_ccq[q].opt()], outs=[hf.opt()])

            for _rep in range(reps):
                hfq = hfq_r[_rep]
                prd_out = prd_out_r[_rep]
                # layer-1 input h~ = dinv * x precomputed on host
                nc.sync.dma_start(ht_sb[:].rearrange("p b e -> p (b e)"),
                                  ht0_d.ap())
                for q in range(NWIN):
                    emit_cc(q, hfq[0][q])

                poolT = fin_ps.tile([H, P], f32, tag="poolT")
                call_no = 0
                for l in range(3):
                    last = l == 2

                    call_tiles = {}
                    layer_call0 = call_no
                    for sg in range(nsg):
                        for (ww, t0, t1) in sg_w_ranges[sg]:
                            src_win = hfq[l][ww][:].rearrange(
                                "p (b e) -> (p b) e", e=EL)
                            t = t0
                            while t < t1:
                                tc_ = min(TMAX, t1 - t)
                                if "gather" not in skip:
                                    mout = msgs_tp.tile(
                                        [P, TMAX // P, EL], bf16,
                                        tag="mc", name="mc")
                                    call_tiles[call_no - layer_call0] = mout
                                    nc.gpsimd.dma_gather(
                                        out_ap=mout[:][:, 0:tc_ // P, :],
                                        in_ap=src_win,
                                        idxs_ap=eidx_sb[:][:, t // 16:
                                                           (t + tc_) // 16],
                                        num_idxs=tc_, num_idxs_reg=tc_,
                                        elem_size=EL, queue_num=call_no % NQ)
                                call_no += 1
                                t += tc_
                        for bi in range(sg * pp["sgblk"],
                                        (sg + 1) * pp["sgblk"]):
                            if "matmul" in skip:
                                continue
                            aggT = agg_ps.tile([H, P], f32, tag="agg",
                                               name="agg")
                            nmm = int(Kmax[bi].sum()) + 1
                            imm = 0
                            # self-loop: aggT += ht_blockT (transpose by I)
                            nc.tensor.matmul(
                                aggT[:], lhsT=ht_sb[:][:, bi, 0:H],
                                rhs=ident_bf[:], start=True, stop=False)
                            imm += 1
                            for ww in range(4):
                                kb = int(Kmax[bi, ww])
                                if kb == 0:
                                    continue
                                col = int(chunk_pos[bi, ww])
                                MB = m_tp.tile([P, kb * P], bf16, tag="MB",
                                               name="MB")
                                nc.vector.tensor_tensor(
                                    out=MB[:].rearrange("p (c q) -> p c q",
                                                        q=P),
                                    in0=edst_sb[:][:, col:col + kb]
                                        .to_broadcast([P, kb, P]),
                                    in1=iota_bf[:][:, None, :]
                                        .to_broadcast([P, kb, P]),
                                    op=AL.is_equal)
                                for j in range(kb):
                                    k_call, slot = chunk_to_call(col + j)
                                    if "gather" in skip:
                                        lhsT_mm = iota_bf[:][:, 0:H]
                                    else:
                                        lhsT_mm = call_tiles[k_call][:][
                                            :, slot, 0:H]
                                    nc.tensor.matmul(
                                        aggT[:],
                                        lhsT=lhsT_mm,
                                        rhs=MB[:][:, j * P:(j + 1) * P],
                                        start=False, stop=(imm == nmm - 1))
                                    imm += 1
                            sT = s_tp.tile([H, P], f32, tag="sT", name="sT")
                            nc.scalar.copy(out=sT[:], in_=aggT[:])
                            outb = out_ps.tile([P, H], f32, tag="outb",
                                               name="outb")
                            nc.tensor.matmul(outb[:], lhsT=sT[:],
                                             rhs=w_sb[l][:],
                                             start=True, stop=True)
                            dcol = dinv_sb[:][:, bi:bi + 1]
                            t1_ = e_tp.tile([P, H], f32, tag="t1", name="t1")
                            nc.vector.tensor_scalar(
                                out=t1_[:], in0=outb[:], scalar1=dcol,
                                scalar2=None, op0=AL.mult)
                            if not last:
                                t2 = e_tp.tile([P, H], f32, tag="t2",
                                               name="t2")
                                nc.vector.tensor_tensor(
                                    out=t2[:], in0=t1_[:], in1=bias_sb[l][:],
                                    op=AL.add)
                                nc.vector.tensor_scalar(
                                    out=ht_sb[:][:, bi, 0:H], in0=t2[:],
                                    scalar1=0.0, scalar2=dcol,
                                    op0=AL.max, op1=AL.mult)
                                if (bi + 1) % QB == 0:
                                    emit_cc(bi // QB, hfq[l + 1][bi // QB])
                            else:
                                # h3 block + fused mean-pool accumulation
                                h3b = e_tp.tile([P, H], f32, tag="h3b",
                                                name="h3b")
                                nc.vector.tensor_tensor(
                                    out=h3b[:], in0=t1_[:], in1=bias_sb[l][:],
                                    op=AL.add)
                                Mg = m_tp.tile([P, P], f32, tag="Mg",
                                               name="Mg")
                                nc.vector.tensor_scalar(
                                    out=Mg[:], in0=iota_f[:],
                                    scalar1=batg_sb[:][:, bi:bi + 1],
                                    scalar2=None, op0=AL.is_equal)
                                nc.tensor.matmul(
                                    poolT[:], lhsT=h3b[:], rhs=Mg[:],
                                    start=(bi == 0), stop=(bi == nblk - 1))

                if "matmul" in skip:
                    continue
                poolT_sb = s_tp.tile([H, P], f32, tag="poolTs")
                nc.vector.tensor_copy(out=poolT_sb[:], in_=poolT[:])
                nc.sync.dma_start(prd_in[:], poolT_sb[:])
                if "collective" not in skip:
                    nc.gpsimd.collective_compute(
                        "AllReduce", AL.add, replica_groups=RG,
                        ins=[prd_in.opt()], outs=[prd_out.opt()])
                poolF = s_tp.tile([H, P], f32, tag="poolF")
                nc.sync.dma_start(poolF[:], prd_out[:])
                fin = fin_ps.tile([P, C], f32, tag="fin")
                nc.tensor.matmul(fin[:], lhsT=poolF[:], rhs=wl_sb[:],
                                 start=True, stop=True)
                outf = e_tp.tile([P, C], f32, tag="outf")
                nc.vector.tensor_scalar(out=outf[:], in0=fin[:],
                                        scalar1=cinv_sb[:], scalar2=None,
                                        op0=AL.mult)
                outf2 = e_tp.tile([P, C], f32, tag="outf2")
                nc.vector.tensor_tensor(out=outf2[:], in0=outf[:],
                                        in1=biasl_sb[:], op=AL.add)
                nc.sync.dma_start(out_d.ap()[:, :], outf2[:][:G, :])

    nc.compile()
    return nc


def make_in_maps(pp, weights, G, n_cores=NCORES):
    W1, b1, W2, b2, W3, b3, Wl, bl, counts = weights
    H = pp["H"]
    C = np.asarray(Wl).shape[1]
    bf = ml_dtypes.bfloat16
    iota_row = np.arange(P, dtype=np.float32)
    iota_bf = np.ascontiguousarray(np.broadcast_to(iota_row, (P, P))).astype(bf)
    iota_f = np.ascontiguousarray(np.broadcast_to(iota_row, (P, P)))
    ident_bf = np.eye(P, dtype=np.float32).astype(bf)
    cinv = np.ones((P, 1), np.float32)
    cinv[:G, 0] = 1.0 / np.maximum(counts, 1.0)
    shared = {
        "iota_bf": iota_bf, "iota_f": iota_f, "ident_bf": ident_bf,
        "w0": np.asarray(W1, np.float32), "w1": np.asarray(W2, np.float32),
        "w2": np.asarray(W3, np.float32),
        "bias0": np.ascontiguousarray(np.broadcast_to(b1, (P, H))).astype(np.float32),
        "bias1": np.ascontiguousarray(np.broadcast_to(b2, (P, H))).astype(np.float32),
        "bias2": np.ascontiguousarray(np.broadcast_to(b3, (P, H))).astype(np.float32),
        "wl": np.asarray(Wl, np.float32),
        "biasl": np.ascontiguousarray(np.broadcast_to(bl, (P, C))).astype(np.float32),
        "cinv": cinv,
    }
    maps = []
    for c in range(n_cores):
        m = dict(shared)
        m["eidx"] = pp["eidx16"][c]
        m["edst"] = pp["edst"][c].astype(bf)
        m["ht0"] = pp["ht0"][c].astype(bf)
        m["dinv"] = pp["dinv"][c]
        m["batg"] = pp["batg"][c]
        maps.append(m)
    return maps


LAST_RESULT = None
LAST_NC = None
LAST_IN_MAPS = None
LAST_PP = None


def kernel(x, edge_index, batch, W1, b1, W2, b2, W3, b3, Wl, bl, **run_kwargs):
    """Full-input entry point. Shards across 8 cores, runs on HW, gathers."""
    global LAST_RESULT, LAST_NC, LAST_IN_MAPS, LAST_PP
    from concourse.bass_utils import run_bass_kernel_spmd

    x = np.asarray(x, np.float32)
    edge_index = np.asarray(edge_index)
    batch = np.asarray(batch)
    G = G_FULL
    C = np.asarray(Wl).shape[1]

    pp = preprocess(x, edge_index, batch)
    counts = np.bincount(batch.astype(np.int64), minlength=G).astype(np.float32)
    nc = build_nc(pp, G, C)
    in_maps = make_in_maps(pp, (W1, b1, W2, b2, W3, b3, Wl, bl, counts), G)
    res = run_bass_kernel_spmd(nc, in_maps, core_ids=list(range(NCORES)),
                               **run_kwargs)
    LAST_RESULT, LAST_NC, LAST_IN_MAPS, LAST_PP = res, nc, in_maps, pp
    return res.results[0]["out"].astype(np.float32)
